# revision 21
# baseline (speedup 1.0000x reference)
"""Trainium2 Bass kernel for nn_Attention_62620623176132.

Multi-head causal attention with RoPE (LLaMA-style), B=2, S=2048, D=2048,
H=16 heads of HD=128, fp32 reference.

Sharding (hardcoded): 8 cores = 2-way data parallel over batch x 4-way
tensor parallel over heads (4 heads per core). Each core computes its 4
heads' Q/K/V projections, attention, and a partial output projection
(rows of wo for its heads); the host sums the 4 partials per batch.

Device algorithm (per core, all matmuls bf16 with fp32 PSUM accumulation):
  - x^T kept SBUF-resident; Q^T/K^T computed per head in [HD, S] layout,
    V in [S, dv] layout (so no transposes are ever needed).
  - RoPE via host-side even/odd column permutation of wq/wk: rotation
    pairs land in partition halves; 3 DVE tensor ops + 2 swap copies.
  - Scores computed transposed: sT[kt, qt] = kT . qT, so exp(sT) feeds
    the PV matmul directly as the moving operand.
  - Softmax denominators via an all-ones stationary matmul (broadcasts
    column sums to all partitions); normalization fused into the
    PSUM->SBUF copy of the attention output.
  - Causality: score tiles above the diagonal are skipped; band tiles
    are restricted to their unmasked columns, and the diagonal square
    gets -1e9 added in PSUM by one extra matmul (identity x triangle),
    so masking costs no vector-engine work at all.
  - Projections for head h+1 are emitted in the middle of head h's
    attention so the serial RoPE chain never stalls the PE.
"""

import math

import numpy as np
import concourse.tile as tile
import concourse.mybir as mybir
from concourse import bacc
from concourse.bass import ts
from concourse.bass_utils import run_bass_kernel_spmd

B, S, D, H, HD = 2, 2048, 2048, 16, 128
P = 128
NCORES = 8
TP = 4                # head-parallel groups
HPC = H // TP         # heads per core = 4
DVC = HPC * HD        # 512 v-dims per core
KC = D // P           # 16 contraction chunks
NT = S // P           # 16 token tiles of 128
NQ = S // 512         # 4 query chunks of 512
F16 = mybir.dt.float16
F32 = mybir.dt.float32
NPF16 = np.float16
MASK_NEG = -60000.0
SCALE = 1.0 / math.sqrt(HD)
EXP = mybir.ActivationFunctionType.Exp

_cache: dict = {}


def _build(mask_mode: str):
    """Build + compile the SPMD program. mask_mode: 'causal'|'none'|'general'."""
    nc = bacc.Bacc("TRN2", target_bir_lowering=False, debug=False,
                   num_devices=NCORES)

    def din(name, shape, dt=F16):
        return nc.dram_tensor(name, shape, dt, kind="ExternalInput").ap()

    xT_d = din("xT", [P, NQ, KC, 512])
    wq_d = din("wq", [P, HPC, KC, HD])
    wk_d = din("wk", [P, HPC, KC, HD])
    wv_d = din("wv", [P, KC, DVC])
    wo_d = din("wo", [P, HPC, D])
    c2_d = din("c2", [P, S])
    s2n_d = din("s2n", [P, S])
    ones_d = din("ones", [P, P])
    if mask_mode == "causal":
        eye_d = din("eye", [P, P])
        mtri_d = din("mtri", [P, P])
    elif mask_mode == "general":
        msk_d = din("expm", [P, NT, S])
    out_d = nc.dram_tensor("out", [P, NT, D], mybir.dt.float16,
                           kind="ExternalOutput").ap()

    with tile.TileContext(nc) as tc:
        with tc.tile_pool(name="static", bufs=1) as st, \
             tc.tile_pool(name="w1", bufs=1) as w1, \
             tc.tile_pool(name="w2", bufs=2) as w2, \
             tc.tile_pool(name="et", bufs=4) as etp, \
             tc.tile_pool(name="ac", bufs=2) as accp, \
             tc.tile_pool(name="fo", bufs=4) as fop, \
             tc.tile_pool(name="pj", bufs=2, space="PSUM") as pjp:

            # ---- static tensors -------------------------------------------
            xT = st.tile([P, NQ, KC, 512], F16, tag="xT")
            wv_sb = st.tile([P, KC, DVC], F16, tag="wv")
            wo_sb = st.tile([P, HPC, D], F16, tag="wo")
            c2 = st.tile([P, S], F16, tag="c2")
            s2n = st.tile([P, S], F16, tag="s2n")
            ones_sb = st.tile([P, P], F16, tag="ones")
            V_sb = st.tile([P, NT, DVC], F16, tag="V")
            OT_sb = st.tile([P, HPC, S], F16, tag="OT")
            if mask_mode == "causal":
                eye_sb = st.tile([P, P], F16, tag="eye")
                mtri_sb = st.tile([P, P], F16, tag="mtri")

            # head-0 weights first (small), then interleaved wv/xT chunks so
            # the V-phase matmuls can start as soon as chunk 0 lands.
            wq_h = w1.tile([P, KC, HD], F16, tag="wqh")
            wk_h = w1.tile([P, KC, HD], F16, tag="wkh")
            for g in range(4):
                nc.sync.dma_start(wq_h[:, ts(g, 4), :], wq_d[:, 0, ts(g, 4), :])
                nc.sync.dma_start(xT[:, 0, ts(g, 4), :], xT_d[:, 0, ts(g, 4), :])
            nc.sync.dma_start(wk_h[:], wk_d[:, 0])
            nc.sync.dma_start(wv_sb[:], wv_d)
            nc.sync.dma_start(xT[:, 1, :, :], xT_d[:, 1, :, :])
            nc.sync.dma_start(xT[:, 2, :, :], xT_d[:, 2, :, :])
            nc.sync.dma_start(xT[:, 3, :, :], xT_d[:, 3, :, :])
            nc.sync.dma_start(c2[:], c2_d)
            nc.sync.dma_start(s2n[:], s2n_d)
            nc.sync.dma_start(ones_sb[:], ones_d)
            if mask_mode == "causal":
                nc.sync.dma_start(eye_sb[:], eye_d)
                nc.sync.dma_start(mtri_sb[:], mtri_d)
            nc.sync.dma_start(wo_sb[:], wo_d)

            def proj_half(w_h, raw_tag, rot_tag):
                """One projection (Q or K) + RoPE -> rotated [HD, S] tile."""
                raw = w1.tile([P, S], F16, tag=raw_tag)
                for t in range(NQ):
                    ps = pjp.tile([P, 512], F32, tag="pj")
                    for kc in range(KC):
                        nc.tensor.matmul(ps[:], w_h[:, kc, :],
                                         xT[:, t, kc, :],
                                         start=(kc == 0), stop=(kc == KC - 1))
                    nc.scalar.copy(raw[:, ts(t, 512)], ps[:])
                return rope(raw, rot_tag)

            def load_w(h):
                wq_h = w1.tile([P, KC, HD], F16, tag="wqh")
                nc.sync.dma_start(wq_h[:], wq_d[:, h])
                wk_h = w1.tile([P, KC, HD], F16, tag="wkh")
                nc.sync.dma_start(wk_h[:], wk_d[:, h])
                return wq_h, wk_h

            # head-0 projections first, Q/K interleaved per token chunk so the
            # PE consumes xT chunks as the startup DMAs land; the RoPE chains
            # then run during the V phase.
            def rope(raw, rot_tag):
                swp = w1.tile([P, S], F16, tag="swap")
                nc.vector.tensor_copy(swp[0:64, :], raw[64:128, :])
                nc.vector.tensor_copy(swp[64:128, :], raw[0:64, :])
                rot = w2.tile([P, S], F16, tag=rot_tag)
                nc.vector.tensor_mul(rot[:], raw[:], c2[:])
                nc.vector.tensor_mul(swp[:], swp[:], s2n[:])
                nc.vector.tensor_add(rot[:], rot[:], swp[:])
                return rot

            qraw0 = w1.tile([P, S], F16, tag="qraw")
            kraw0 = w1.tile([P, S], F16, tag="kraw")
            for t in range(NQ):
                for w_h, raw in ((wq_h, qraw0), (wk_h, kraw0)):
                    ps = pjp.tile([P, 512], F32, tag="pj")
                    for kc in range(KC):
                        nc.tensor.matmul(ps[:], w_h[:, kc, :], xT[:, t, kc, :],
                                         start=(kc == 0), stop=(kc == KC - 1))
                    nc.scalar.copy(raw[:, ts(t, 512)], ps[:])
                # V projection for this token chunk keeps the PE busy while
                # the next xT chunk is still streaming in
                for ti in range(4 * t, 4 * t + 4):
                    ps = pjp.tile([P, 512], F32, tag="pj")
                    for kc in range(KC):
                        nc.tensor.matmul(ps[:], xT[:, t, kc, ts(ti % 4, P)],
                                         wv_sb[:, kc, :],
                                         start=(kc == 0), stop=(kc == KC - 1))
                    nc.scalar.copy(V_sb[:, ti, :], ps[:])
            rots = {0: (rope(qraw0, "qrot"), rope(kraw0, "krot"))}

            flip = [False]

            def emit_f(qi, pool, scalar_only=False):
                for nn in range(D // 512):
                    ps = pool.tile([P, 512], F32, tag="pj")
                    for hh in range(HPC):
                        nc.tensor.matmul(ps[:], OT_sb[:, hh, ts(qi, P)],
                                         wo_sb[:, hh, ts(nn, 512)],
                                         start=(hh == 0), stop=(hh == HPC - 1))
                    f_sb = fop.tile([P, 512], F16, tag="fsb")
                    # keep the copies off the Vector engine when F is inlined
                    # into attention: they would head-of-line-block the acc
                    # adds in DVE's in-order queue
                    if flip[0] and not scalar_only:
                        nc.vector.tensor_copy(f_sb[:], ps[:])
                    else:
                        nc.scalar.copy(f_sb[:], ps[:])
                    flip[0] = not flip[0]
                    nc.sync.dma_start(out_d[:, qi, ts(nn, 512)], f_sb[:])

            # ---- attention, with next head's projections interleaved ------
            with tc.tile_pool(name="sp", bufs=3, space="PSUM") as stp, \
                 tc.tile_pool(name="op", bufs=2, space="PSUM") as opp, \
                 tc.tile_pool(name="rp", bufs=1, space="PSUM") as rpp:
                for h in range(HPC):
                    qrot, krot = rots.pop(h)
                    for qc in range(NQ):
                        o_ps = opp.tile([P, 512], F32, tag="o")
                        r_ps = rpp.tile([P, 512], F32, tag="r")
                        nkt = 4 * (qc + 1) if mask_mode == "causal" else NT
                        nfull = 4 * qc if mask_mode == "causal" else 0
                        # previous chunk's output-projection tiles, spread one
                        # per kt iteration so their PSUM->SBUF copies pace
                        # evenly through both engines' queues
                        fq = (list(range(4 * (qc - 1), 4 * qc))
                              if h == HPC - 1 and mask_mode == "causal" and qc > 0
                              else [])
                        # full-width tiles accumulate on the DVE into `acc`;
                        # one ones-matmul on the sum replaces one per tile.
                        acc = first_e = None
                        for kt in range(nkt):
                            band = mask_mode == "causal" and kt >= nfull
                            off = 128 * (kt - nfull) if band else 0
                            s_ps = stp.tile([P, 512], F32, tag="s")
                            nc.tensor.matmul(
                                s_ps[:, off:], krot[:, ts(kt, P)],
                                qrot[:, 512 * qc + off: 512 * (qc + 1)],
                                start=True, stop=not band)
                            if band:
                                nc.tensor.matmul(
                                    s_ps[:, off:off + 128], eye_sb[:], mtri_sb[:],
                                    start=False, stop=True)
                            eT = etp.tile([P, 512], F16, tag="e")
                            nc.scalar.activation(eT[:, off:], s_ps[:, off:], EXP,
                                                 scale=SCALE)
                            if mask_mode == "general":
                                em = etp.tile([P, 512], F16, tag="em")
                                nc.sync.dma_start(em[:], msk_d[:, kt, ts(qc, 512)])
                                nc.gpsimd.tensor_mul(eT[:], eT[:], em[:])
                            nc.tensor.matmul(o_ps[:, off:],
                                             V_sb[:, kt, ts(h, HD)], eT[:, off:],
                                             start=(kt == 0), stop=(kt == nkt - 1))
                            if mask_mode != "causal":
                                nc.tensor.matmul(r_ps[:], ones_sb[:], eT[:],
                                                 start=(kt == 0),
                                                 stop=(kt == nkt - 1))
                            elif not band:
                                if first_e is not None:
                                    acc = accp.tile([P, 512], F16, tag="acc")
                                    nc.vector.tensor_add(acc[:], first_e[:], eT[:])
                                    first_e = None
                                elif acc is not None:
                                    nc.vector.tensor_add(acc[:], acc[:], eT[:])
                                else:
                                    first_e = eT
                            elif acc is None and first_e is None:
                                acc = accp.tile([P, 512], F16, tag="acc")
                                nc.vector.tensor_copy(acc[:], eT[:])
                            elif first_e is not None:
                                acc = accp.tile([P, 512], F16, tag="acc")
                                nc.vector.tensor_add(acc[:], first_e[:], eT[:])
                                first_e = None
                            else:
                                nc.vector.tensor_add(acc[:, off:], acc[:, off:],
                                                     eT[:, off:])
                            if fq and kt % 3 == 2:
                                emit_f(fq.pop(0), pjp)
                        if acc is not None:
                            nc.tensor.matmul(r_ps[:], ones_sb[:], acc[:],
                                             start=True, stop=True)
                        rinv = fop.tile([P, 512], F32, tag="rinv")
                        nc.vector.reciprocal_approx_fast(out=rinv[:], in_=r_ps[:])
                        nc.vector.tensor_mul(OT_sb[:, h, ts(qc, 512)], o_ps[:],
                                             rinv[:])
                        # pipeline the next head's projections + RoPE; on the
                        # last head, stream the output projection instead
                        if h + 1 < HPC and qc == 0:
                            wq_n, wk_n = load_w(h + 1)
                            rots[h + 1] = (proj_half(wq_n, "qraw", "qrot"),
                                           proj_half(wk_n, "kraw", "krot"))
                        elif fq:
                            # whatever didn't fit between kt iterations
                            for qi in fq:
                                emit_f(qi, pjp)
                    if h == HPC - 1 and mask_mode == "causal":
                        for qi in range(4 * (NQ - 1), NT):
                            emit_f(qi, pjp)

            # ---- output projection for non-causal modes (causal streams it
            # inside the last head's attention) --------------------------------
            if mask_mode != "causal":
                with tc.tile_pool(name="fp", bufs=6, space="PSUM") as fpp:
                    for qi in range(NT):
                        emit_f(qi, fpp)

    nc.compile()
    return nc


def _get_program(mask_mode: str):
    if mask_mode not in _cache:
        _cache[mask_mode] = _build(mask_mode)
    return _cache[mask_mode]


def _detect_mask_mode(mask: np.ndarray) -> str:
    m = mask.reshape(S, S)
    iu = np.triu_indices(S, 1)
    upper = m[iu]
    lower_ok = np.max(np.abs(np.tril(m))) == 0.0
    if lower_ok and upper.size and np.all(upper <= -1e8):
        return "causal"
    if np.max(np.abs(m)) == 0.0:
        return "none"
    return "general"


def _prep_inputs(x, wq, wk, wv, wo, freqs_cos, freqs_sin, mask, mask_mode):
    """Build the 8 per-core input maps (host-side sharding + layout)."""
    # within-head even/odd permutation so RoPE pairs land in partition halves
    perm = np.concatenate([np.arange(0, HD, 2), np.arange(1, HD, 2)])

    cosT = freqs_cos.T.astype(np.float32)          # [64, S]
    sinT = freqs_sin.T.astype(np.float32)
    c2 = np.concatenate([cosT, cosT], 0).astype(NPF16)     # [128, S]
    s2n = np.concatenate([-sinT, sinT], 0).astype(NPF16)
    ones = np.ones((P, P), NPF16)

    common = {"c2": c2, "s2n": s2n, "ones": ones}
    if mask_mode == "causal":
        common["eye"] = np.eye(P, dtype=NPF16)
        pp, ff = np.meshgrid(np.arange(P), np.arange(P), indexing="ij")
        common["mtri"] = np.where(pp > ff, MASK_NEG, 0.0).astype(NPF16)
    elif mask_mode == "general":
        m = mask.reshape(S, S).astype(np.float32)
        # eT[kt_tok, qt_tok] is multiplied by exp(SCALE * mask[qt_tok, kt_tok])
        expm = np.exp(SCALE * m.T).astype(NPF16)            # [k_tok, q_tok]
        common["expm"] = np.ascontiguousarray(
            expm.reshape(NT, P, S).transpose(1, 0, 2))

    xT_by_b = []
    for b in range(B):
        xT = np.ascontiguousarray(
            x[b].T.reshape(KC, P, NQ, 512).transpose(1, 2, 0, 3)).astype(NPF16)
        xT_by_b.append(xT)

    in_maps = []
    for c in range(NCORES):
        b, g = divmod(c, TP)
        heads = range(g * HPC, (g + 1) * HPC)
        cols_qk = np.concatenate([h * HD + perm for h in heads])
        cols_v = np.concatenate([np.arange(h * HD, (h + 1) * HD) for h in heads])

        wq_c = wq[:, cols_qk].reshape(KC, P, HPC, HD).transpose(1, 2, 0, 3)
        wk_c = wk[:, cols_qk].reshape(KC, P, HPC, HD).transpose(1, 2, 0, 3)
        wv_c = wv[:, cols_v].reshape(KC, P, DVC).transpose(1, 0, 2)
        wo_c = wo[cols_v, :].reshape(HPC, P, D).transpose(1, 0, 2)

        im = dict(common)
        im["xT"] = xT_by_b[b]
        im["wq"] = np.ascontiguousarray(wq_c).astype(NPF16)
        im["wk"] = np.ascontiguousarray(wk_c).astype(NPF16)
        im["wv"] = np.ascontiguousarray(wv_c).astype(NPF16)
        im["wo"] = np.ascontiguousarray(wo_c).astype(NPF16)
        in_maps.append(im)
    return in_maps


def run(inputs: dict, **spmd_kwargs):
    """Run on hardware; returns (output [B,S,D] fp32, BassKernelResults)."""
    x = np.asarray(inputs["x"], np.float32)
    wq = np.asarray(inputs["wq"], np.float32)
    wk = np.asarray(inputs["wk"], np.float32)
    wv = np.asarray(inputs["wv"], np.float32)
    wo = np.asarray(inputs["wo"], np.float32)
    fc = np.asarray(inputs["freqs_cos"], np.float32)
    fs = np.asarray(inputs["freqs_sin"], np.float32)
    mask = np.asarray(inputs["mask"], np.float32)

    mask_mode = _detect_mask_mode(mask)
    nc = _get_program(mask_mode)
    in_maps = _prep_inputs(x, wq, wk, wv, wo, fc, fs, mask, mask_mode)
    res = run_bass_kernel_spmd(nc, in_maps, core_ids=list(range(NCORES)),
                               **spmd_kwargs)

    out = np.zeros((B, S, D), np.float32)
    for c in range(NCORES):
        b = c // TP
        part = res.results[c]["out"].astype(np.float32)   # [P, NT, D]
        out[b] += part.transpose(1, 0, 2).reshape(S, D)
    return out, res


def kernel(**inputs) -> np.ndarray:
    out, _ = run(inputs)
    return out


# revision 22
# speedup vs baseline: 1.0057x; 1.0057x over previous
"""Trainium2 Bass kernel for nn_Attention_62620623176132.

Multi-head causal attention with RoPE (LLaMA-style), B=2, S=2048, D=2048,
H=16 heads of HD=128, fp32 reference.

Sharding (hardcoded): 8 cores = 2-way data parallel over batch x 4-way
tensor parallel over heads (4 heads per core). Each core computes its 4
heads' Q/K/V projections, attention, and a partial output projection
(rows of wo for its heads); the host sums the 4 partials per batch.

Device algorithm (per core, all matmuls bf16 with fp32 PSUM accumulation):
  - x^T kept SBUF-resident; Q^T/K^T computed per head in [HD, S] layout,
    V in [S, dv] layout (so no transposes are ever needed).
  - RoPE via host-side even/odd column permutation of wq/wk: rotation
    pairs land in partition halves; 3 DVE tensor ops + 2 swap copies.
  - Scores computed transposed: sT[kt, qt] = kT . qT, so exp(sT) feeds
    the PV matmul directly as the moving operand.
  - Softmax denominators via an all-ones stationary matmul (broadcasts
    column sums to all partitions); normalization fused into the
    PSUM->SBUF copy of the attention output.
  - Causality: score tiles above the diagonal are skipped; band tiles
    are restricted to their unmasked columns, and the diagonal square
    gets -1e9 added in PSUM by one extra matmul (identity x triangle),
    so masking costs no vector-engine work at all.
  - Projections for head h+1 are emitted in the middle of head h's
    attention so the serial RoPE chain never stalls the PE.
"""

import math

import numpy as np
import concourse.tile as tile
import concourse.mybir as mybir
from concourse import bacc
from concourse.bass import ts
from concourse.bass_utils import run_bass_kernel_spmd

B, S, D, H, HD = 2, 2048, 2048, 16, 128
P = 128
NCORES = 8
TP = 4                # head-parallel groups
HPC = H // TP         # heads per core = 4
DVC = HPC * HD        # 512 v-dims per core
KC = D // P           # 16 contraction chunks
NT = S // P           # 16 token tiles of 128
NQ = S // 512         # 4 query chunks of 512
F16 = mybir.dt.float16
F32 = mybir.dt.float32
NPF16 = np.float16
MASK_NEG = -60000.0
SCALE = 1.0 / math.sqrt(HD)
EXP = mybir.ActivationFunctionType.Exp

_cache: dict = {}


def _build(mask_mode: str):
    """Build + compile the SPMD program. mask_mode: 'causal'|'none'|'general'."""
    nc = bacc.Bacc("TRN2", target_bir_lowering=False, debug=False,
                   num_devices=NCORES)

    def din(name, shape, dt=F16):
        return nc.dram_tensor(name, shape, dt, kind="ExternalInput").ap()

    xT_d = din("xT", [P, NQ, KC, 512])
    wq_d = din("wq", [P, HPC, KC, HD])
    wk_d = din("wk", [P, HPC, KC, HD])
    wv_d = din("wv", [P, KC, DVC])
    wo_d = din("wo", [P, HPC, D])
    c2_d = din("c2", [P, S])
    s2n_d = din("s2n", [P, S])
    ones_d = din("ones", [P, P])
    if mask_mode == "causal":
        eye_d = din("eye", [P, P])
        mtri_d = din("mtri", [P, P])
    elif mask_mode == "general":
        msk_d = din("expm", [P, NT, S])
    out_d = nc.dram_tensor("out", [P, NT, D], mybir.dt.float16,
                           kind="ExternalOutput").ap()

    with tile.TileContext(nc) as tc:
        with tc.tile_pool(name="static", bufs=1) as st, \
             tc.tile_pool(name="w1", bufs=1) as w1, \
             tc.tile_pool(name="w2", bufs=2) as w2, \
             tc.tile_pool(name="et", bufs=6) as etp, \
             tc.tile_pool(name="ac", bufs=3) as accp, \
             tc.tile_pool(name="fo", bufs=4) as fop, \
             tc.tile_pool(name="pj", bufs=2, space="PSUM") as pjp:

            # ---- static tensors -------------------------------------------
            xT = st.tile([P, NQ, KC, 512], F16, tag="xT")
            wv_sb = st.tile([P, KC, DVC], F16, tag="wv")
            wo_sb = st.tile([P, HPC, D], F16, tag="wo")
            c2 = st.tile([P, S], F16, tag="c2")
            s2n = st.tile([P, S], F16, tag="s2n")
            ones_sb = st.tile([P, P], F16, tag="ones")
            V_sb = st.tile([P, NT, DVC], F16, tag="V")
            OT_sb = st.tile([P, HPC, S], F16, tag="OT")
            if mask_mode == "causal":
                eye_sb = st.tile([P, P], F16, tag="eye")
                mtri_sb = st.tile([P, P], F16, tag="mtri")

            # head-0 weights first (small), then interleaved wv/xT chunks so
            # the V-phase matmuls can start as soon as chunk 0 lands.
            wq_h = w1.tile([P, KC, HD], F16, tag="wqh")
            wk_h = w1.tile([P, KC, HD], F16, tag="wkh")
            for g in range(4):
                nc.sync.dma_start(wq_h[:, ts(g, 4), :], wq_d[:, 0, ts(g, 4), :])
                nc.sync.dma_start(xT[:, 0, ts(g, 4), :], xT_d[:, 0, ts(g, 4), :])
            nc.sync.dma_start(wk_h[:], wk_d[:, 0])
            nc.sync.dma_start(wv_sb[:], wv_d)
            nc.sync.dma_start(xT[:, 1, :, :], xT_d[:, 1, :, :])
            nc.sync.dma_start(xT[:, 2, :, :], xT_d[:, 2, :, :])
            nc.sync.dma_start(xT[:, 3, :, :], xT_d[:, 3, :, :])
            nc.sync.dma_start(c2[:], c2_d)
            nc.sync.dma_start(s2n[:], s2n_d)
            nc.sync.dma_start(ones_sb[:], ones_d)
            if mask_mode == "causal":
                nc.sync.dma_start(eye_sb[:], eye_d)
                nc.sync.dma_start(mtri_sb[:], mtri_d)
            nc.sync.dma_start(wo_sb[:], wo_d)

            def proj_half(w_h, raw_tag, rot_tag):
                """One projection (Q or K) + RoPE -> rotated [HD, S] tile."""
                raw = w1.tile([P, S], F16, tag=raw_tag)
                for t in range(NQ):
                    ps = pjp.tile([P, 512], F32, tag="pj")
                    for kc in range(KC):
                        nc.tensor.matmul(ps[:], w_h[:, kc, :],
                                         xT[:, t, kc, :],
                                         start=(kc == 0), stop=(kc == KC - 1))
                    nc.scalar.copy(raw[:, ts(t, 512)], ps[:])
                return rope(raw, rot_tag)

            def load_w(h):
                wq_h = w1.tile([P, KC, HD], F16, tag="wqh")
                nc.sync.dma_start(wq_h[:], wq_d[:, h])
                wk_h = w1.tile([P, KC, HD], F16, tag="wkh")
                nc.sync.dma_start(wk_h[:], wk_d[:, h])
                return wq_h, wk_h

            # head-0 projections first, Q/K interleaved per token chunk so the
            # PE consumes xT chunks as the startup DMAs land; the RoPE chains
            # then run during the V phase.
            def rope(raw, rot_tag):
                swp = w1.tile([P, S], F16, tag="swap")
                nc.vector.tensor_copy(swp[0:64, :], raw[64:128, :])
                nc.vector.tensor_copy(swp[64:128, :], raw[0:64, :])
                rot = w2.tile([P, S], F16, tag=rot_tag)
                nc.vector.tensor_mul(rot[:], raw[:], c2[:])
                nc.vector.tensor_mul(swp[:], swp[:], s2n[:])
                nc.vector.tensor_add(rot[:], rot[:], swp[:])
                return rot

            qraw0 = w1.tile([P, S], F16, tag="qraw")
            kraw0 = w1.tile([P, S], F16, tag="kraw")
            for t in range(NQ):
                for w_h, raw in ((wq_h, qraw0), (wk_h, kraw0)):
                    ps = pjp.tile([P, 512], F32, tag="pj")
                    for kc in range(KC):
                        nc.tensor.matmul(ps[:], w_h[:, kc, :], xT[:, t, kc, :],
                                         start=(kc == 0), stop=(kc == KC - 1))
                    nc.scalar.copy(raw[:, ts(t, 512)], ps[:])
                # V projection for this token chunk keeps the PE busy while
                # the next xT chunk is still streaming in
                for ti in range(4 * t, 4 * t + 4):
                    ps = pjp.tile([P, 512], F32, tag="pj")
                    for kc in range(KC):
                        nc.tensor.matmul(ps[:], xT[:, t, kc, ts(ti % 4, P)],
                                         wv_sb[:, kc, :],
                                         start=(kc == 0), stop=(kc == KC - 1))
                    nc.scalar.copy(V_sb[:, ti, :], ps[:])
            rots = {0: (rope(qraw0, "qrot"), rope(kraw0, "krot"))}

            flip = [False]

            def emit_f(qi, pool, scalar_only=False):
                for nn in range(D // 512):
                    ps = pool.tile([P, 512], F32, tag="pj")
                    for hh in range(HPC):
                        nc.tensor.matmul(ps[:], OT_sb[:, hh, ts(qi, P)],
                                         wo_sb[:, hh, ts(nn, 512)],
                                         start=(hh == 0), stop=(hh == HPC - 1))
                    f_sb = fop.tile([P, 512], F16, tag="fsb")
                    # keep the copies off the Vector engine when F is inlined
                    # into attention: they would head-of-line-block the acc
                    # adds in DVE's in-order queue
                    if flip[0] and not scalar_only:
                        nc.vector.tensor_copy(f_sb[:], ps[:])
                    else:
                        nc.scalar.copy(f_sb[:], ps[:])
                    flip[0] = not flip[0]
                    nc.sync.dma_start(out_d[:, qi, ts(nn, 512)], f_sb[:])

            # ---- attention, with next head's projections interleaved ------
            with tc.tile_pool(name="sp", bufs=3, space="PSUM") as stp, \
                 tc.tile_pool(name="op", bufs=2, space="PSUM") as opp, \
                 tc.tile_pool(name="rp", bufs=1, space="PSUM") as rpp:
                for h in range(HPC):
                    qrot, krot = rots.pop(h)
                    for qc in range(NQ):
                        o_ps = opp.tile([P, 512], F32, tag="o")
                        r_ps = rpp.tile([P, 512], F32, tag="r")
                        nkt = 4 * (qc + 1) if mask_mode == "causal" else NT
                        nfull = 4 * qc if mask_mode == "causal" else 0
                        # previous chunk's output-projection tiles, spread one
                        # per kt iteration so their PSUM->SBUF copies pace
                        # evenly through both engines' queues
                        fq = (list(range(4 * (qc - 1), 4 * qc))
                              if h == HPC - 1 and mask_mode == "causal" and qc > 0
                              else [])
                        # full-width tiles accumulate on the DVE into `acc`;
                        # one ones-matmul on the sum replaces one per tile.
                        acc = first_e = None
                        for kt in range(nkt):
                            band = mask_mode == "causal" and kt >= nfull
                            off = 128 * (kt - nfull) if band else 0
                            s_ps = stp.tile([P, 512], F32, tag="s")
                            nc.tensor.matmul(
                                s_ps[:, off:], krot[:, ts(kt, P)],
                                qrot[:, 512 * qc + off: 512 * (qc + 1)],
                                start=True, stop=not band)
                            if band:
                                nc.tensor.matmul(
                                    s_ps[:, off:off + 128], eye_sb[:], mtri_sb[:],
                                    start=False, stop=True)
                            eT = etp.tile([P, 512], F16, tag="e")
                            nc.scalar.activation(eT[:, off:], s_ps[:, off:], EXP,
                                                 scale=SCALE)
                            if mask_mode == "general":
                                em = etp.tile([P, 512], F16, tag="em")
                                nc.sync.dma_start(em[:], msk_d[:, kt, ts(qc, 512)])
                                nc.gpsimd.tensor_mul(eT[:], eT[:], em[:])
                            nc.tensor.matmul(o_ps[:, off:],
                                             V_sb[:, kt, ts(h, HD)], eT[:, off:],
                                             start=(kt == 0), stop=(kt == nkt - 1))
                            if mask_mode != "causal":
                                nc.tensor.matmul(r_ps[:], ones_sb[:], eT[:],
                                                 start=(kt == 0),
                                                 stop=(kt == nkt - 1))
                            elif not band:
                                if first_e is not None:
                                    acc = accp.tile([P, 512], F16, tag="acc")
                                    nc.vector.tensor_add(acc[:], first_e[:], eT[:])
                                    first_e = None
                                elif acc is not None:
                                    nc.vector.tensor_add(acc[:], acc[:], eT[:])
                                else:
                                    first_e = eT
                            elif acc is None and first_e is None:
                                acc = accp.tile([P, 512], F16, tag="acc")
                                nc.vector.tensor_copy(acc[:], eT[:])
                            elif first_e is not None:
                                acc = accp.tile([P, 512], F16, tag="acc")
                                nc.vector.tensor_add(acc[:], first_e[:], eT[:])
                                first_e = None
                            else:
                                nc.vector.tensor_add(acc[:, off:], acc[:, off:],
                                                     eT[:, off:])
                            if fq and kt % 3 == 2:
                                emit_f(fq.pop(0), pjp)
                        if acc is not None:
                            nc.tensor.matmul(r_ps[:], ones_sb[:], acc[:],
                                             start=True, stop=True)
                        rinv = fop.tile([P, 512], F32, tag="rinv")
                        nc.vector.reciprocal_approx_fast(out=rinv[:], in_=r_ps[:])
                        nc.vector.tensor_mul(OT_sb[:, h, ts(qc, 512)], o_ps[:],
                                             rinv[:])
                        # pipeline the next head's projections + RoPE; on the
                        # last head, stream the output projection instead
                        if h + 1 < HPC and qc == 0:
                            wq_n, wk_n = load_w(h + 1)
                            rots[h + 1] = (proj_half(wq_n, "qraw", "qrot"),
                                           proj_half(wk_n, "kraw", "krot"))
                        elif fq:
                            # whatever didn't fit between kt iterations
                            for qi in fq:
                                emit_f(qi, pjp)
                    if h == HPC - 1 and mask_mode == "causal":
                        for qi in range(4 * (NQ - 1), NT):
                            emit_f(qi, pjp)

            # ---- output projection for non-causal modes (causal streams it
            # inside the last head's attention) --------------------------------
            if mask_mode != "causal":
                with tc.tile_pool(name="fp", bufs=6, space="PSUM") as fpp:
                    for qi in range(NT):
                        emit_f(qi, fpp)

    nc.compile()
    return nc


def _get_program(mask_mode: str):
    if mask_mode not in _cache:
        _cache[mask_mode] = _build(mask_mode)
    return _cache[mask_mode]


def _detect_mask_mode(mask: np.ndarray) -> str:
    m = mask.reshape(S, S)
    iu = np.triu_indices(S, 1)
    upper = m[iu]
    lower_ok = np.max(np.abs(np.tril(m))) == 0.0
    if lower_ok and upper.size and np.all(upper <= -1e8):
        return "causal"
    if np.max(np.abs(m)) == 0.0:
        return "none"
    return "general"


def _prep_inputs(x, wq, wk, wv, wo, freqs_cos, freqs_sin, mask, mask_mode):
    """Build the 8 per-core input maps (host-side sharding + layout)."""
    # within-head even/odd permutation so RoPE pairs land in partition halves
    perm = np.concatenate([np.arange(0, HD, 2), np.arange(1, HD, 2)])

    cosT = freqs_cos.T.astype(np.float32)          # [64, S]
    sinT = freqs_sin.T.astype(np.float32)
    c2 = np.concatenate([cosT, cosT], 0).astype(NPF16)     # [128, S]
    s2n = np.concatenate([-sinT, sinT], 0).astype(NPF16)
    ones = np.ones((P, P), NPF16)

    common = {"c2": c2, "s2n": s2n, "ones": ones}
    if mask_mode == "causal":
        common["eye"] = np.eye(P, dtype=NPF16)
        pp, ff = np.meshgrid(np.arange(P), np.arange(P), indexing="ij")
        common["mtri"] = np.where(pp > ff, MASK_NEG, 0.0).astype(NPF16)
    elif mask_mode == "general":
        m = mask.reshape(S, S).astype(np.float32)
        # eT[kt_tok, qt_tok] is multiplied by exp(SCALE * mask[qt_tok, kt_tok])
        expm = np.exp(SCALE * m.T).astype(NPF16)            # [k_tok, q_tok]
        common["expm"] = np.ascontiguousarray(
            expm.reshape(NT, P, S).transpose(1, 0, 2))

    xT_by_b = []
    for b in range(B):
        xT = np.ascontiguousarray(
            x[b].T.reshape(KC, P, NQ, 512).transpose(1, 2, 0, 3)).astype(NPF16)
        xT_by_b.append(xT)

    in_maps = []
    for c in range(NCORES):
        b, g = divmod(c, TP)
        heads = range(g * HPC, (g + 1) * HPC)
        cols_qk = np.concatenate([h * HD + perm for h in heads])
        cols_v = np.concatenate([np.arange(h * HD, (h + 1) * HD) for h in heads])

        wq_c = wq[:, cols_qk].reshape(KC, P, HPC, HD).transpose(1, 2, 0, 3)
        wk_c = wk[:, cols_qk].reshape(KC, P, HPC, HD).transpose(1, 2, 0, 3)
        wv_c = wv[:, cols_v].reshape(KC, P, DVC).transpose(1, 0, 2)
        wo_c = wo[cols_v, :].reshape(HPC, P, D).transpose(1, 0, 2)

        im = dict(common)
        im["xT"] = xT_by_b[b]
        im["wq"] = np.ascontiguousarray(wq_c).astype(NPF16)
        im["wk"] = np.ascontiguousarray(wk_c).astype(NPF16)
        im["wv"] = np.ascontiguousarray(wv_c).astype(NPF16)
        im["wo"] = np.ascontiguousarray(wo_c).astype(NPF16)
        in_maps.append(im)
    return in_maps


def run(inputs: dict, **spmd_kwargs):
    """Run on hardware; returns (output [B,S,D] fp32, BassKernelResults)."""
    x = np.asarray(inputs["x"], np.float32)
    wq = np.asarray(inputs["wq"], np.float32)
    wk = np.asarray(inputs["wk"], np.float32)
    wv = np.asarray(inputs["wv"], np.float32)
    wo = np.asarray(inputs["wo"], np.float32)
    fc = np.asarray(inputs["freqs_cos"], np.float32)
    fs = np.asarray(inputs["freqs_sin"], np.float32)
    mask = np.asarray(inputs["mask"], np.float32)

    mask_mode = _detect_mask_mode(mask)
    nc = _get_program(mask_mode)
    in_maps = _prep_inputs(x, wq, wk, wv, wo, fc, fs, mask, mask_mode)
    res = run_bass_kernel_spmd(nc, in_maps, core_ids=list(range(NCORES)),
                               **spmd_kwargs)

    out = np.zeros((B, S, D), np.float32)
    for c in range(NCORES):
        b = c // TP
        part = res.results[c]["out"].astype(np.float32)   # [P, NT, D]
        out[b] += part.transpose(1, 0, 2).reshape(S, D)
    return out, res


def kernel(**inputs) -> np.ndarray:
    out, _ = run(inputs)
    return out
